# revision 1
# baseline (speedup 1.0000x reference)
"""Trainium2 Bass kernel for nn_MaxExtractor (masked pairwise-IoU max + union max).

Contract: kernel(**inputs) takes FULL unsharded inputs, returns the FULL [2]
output. Internally shards the batch dim (8 images) across 8 NeuronCores, one
image per core; each core computes [max_prob_t, iou_max_of_its_image]; the
host gathers and averages the per-image iou scalars.

Per-core algorithm (N=4096 preds, M=2048 gts):
  Only preds with class==0 (~50/4096) matter, so the core first compacts
  person preds on-device: mask -> free-dim cumsum (tensor_tensor_scan) ->
  cross-partition exclusive prefix (triangular matmul) -> ranks -> one-hot
  -> accumulating PE matmuls gather person boxes into [128, 4].
  Slot layout (K=64): partition p holds person (p % 64) and gt-half (p // 64),
  so pairwise tiles are [128, 512] over 2 gt blocks.
  Gt/pred boxes are pre-split on the host into bf16 hi+lo parts (lossless to
  ~2^-18); two accumulating bf16 matmuls reconstruct fp32 in PSUM at the PE's
  1 cycle/row bf16 rate, dodging the 4 cycles/row fp32 tax.
  Pairwise: iw = min(px2,gx2)-max(px1,gx1) etc. via fused DVE ops; instead of
  iou = inter/uni per pair, rank by r = inter/(area_p+area_g+eps) which is a
  monotone transform of iou (iou = r/(1-r)); one reciprocal_approx_fast per
  block replaces per-pair division. Final: partition all-reduce max,
  iou = r*/(1-r*).
"""

import sys

sys.path.insert(0, "/opt/trn_rl_repo")

import contextlib

import numpy as np

import concourse.bacc as bacc
import concourse.mybir as mybir
from concourse import bass_isa
from concourse.tile import TileContext

F32 = mybir.dt.float32
BF16 = mybir.dt.bfloat16
I32 = mybir.dt.int32
Alu = mybir.AluOpType
Act = mybir.ActivationFunctionType

N = 4096  # preds per image
M = 2048  # gts per image
B = 8  # images == cores
U = 4096  # union entries
BIG = 1.0e30
EPS = 1.0e-9
FDB = 512  # gt-block free size for the pairwise phase (1 PSUM bank)
NCH = 32  # pred chunks of 128 (compaction contract dim)


def split_hi_lo(x: np.ndarray):
    """bf16 hi+lo decomposition of fp32 data, exact to ~2^-18 relative."""
    bf16 = mybir.dt.np(BF16)
    hi = x.astype(bf16)
    lo = (x - hi.astype(np.float32)).astype(bf16)
    return hi, lo


def build_kernel(K: int):
    """Build the per-core Bass module. K = person-slot count (64 or 128)."""
    assert K in (64, 128)
    nhalf = 128 // K  # gt-halves packed along partitions
    nblk = M // (nhalf * FDB)  # sequential gt blocks
    GW = 4 * M // nhalf  # gt row width (elems per half)

    nc = bacc.Bacc("TRN2", target_bir_lowering=False, debug=False)

    # packed inputs (host-side interleave): misc = [pred_classes | union_scores
    # | union_classes] as u32 rows; pb_hl/gt_hl = bf16 hi/lo interleaved per box
    misc = nc.dram_tensor("misc", [3, N], mybir.dt.uint32, kind="ExternalInput")
    pb_hl = nc.dram_tensor("pb_hl", [N, 8], BF16, kind="ExternalInput")
    gt_boxes = nc.dram_tensor("gt_boxes", [M, 4], F32, kind="ExternalInput")
    gt_hl = nc.dram_tensor("gt_hl", [M, 8], BF16, kind="ExternalInput")
    out = nc.dram_tensor("out", [2], F32, kind="ExternalOutput")

    # ---- constants baked into the NEFF ----
    # col 0-127: strict lower-tri (partition prefix); col 128-255: one-hot
    # rank targets (slot p matches rank (p % K) + 1) as bf16 pair-packed f32
    tri_np = (np.arange(128)[:, None] < np.arange(128)[None, :]).astype(np.float32)
    iota_np = np.tile(np.arange(1, K + 1, dtype=np.float32), 128 // K)
    iota_bf = np.broadcast_to(iota_np.astype(mybir.dt.np(BF16)), (128, 128))
    iota_as_f32 = np.ascontiguousarray(iota_bf).view(np.uint16).astype(np.uint32)
    merged = np.concatenate(
        [tri_np.view(np.uint32), (iota_as_f32[:, 0::2] | (iota_as_f32[:, 1::2] << 16))],
        axis=1,
    )  # [128, 192] u32: tri | bf16-packed iota
    sel_np = np.zeros((128, 128), dtype=np.float32)
    for p in range(128):
        sel_np[(p // K) % nhalf, p] = 1.0
    sel16_bits = np.ascontiguousarray(sel_np.astype(mybir.dt.np(BF16))).view(np.uint16).astype(np.uint32)
    sel16_packed = sel16_bits[:, 0::2] | (sel16_bits[:, 1::2] << 16)
    merged = np.concatenate(
        [merged, sel_np.view(np.uint32), sel16_packed], axis=1
    )  # [128, 384] u32: tri | iota | sel_f32 | sel_bf16
    const_merged = nc.inline_tensor(merged.astype(np.uint32), "consts")

    with TileContext(nc) as tc:
        ctx = contextlib.ExitStack()
        with ctx:
            const_pool = ctx.enter_context(tc.tile_pool(name="const", bufs=1))
            sb = ctx.enter_context(tc.tile_pool(name="sbuf", bufs=1))
            wrk = ctx.enter_context(tc.tile_pool(name="wrk", bufs=2))
            ohp = ctx.enter_context(tc.tile_pool(name="ohp", bufs=32))
            small = ctx.enter_context(tc.tile_pool(name="small", bufs=1))
            ps_g = ctx.enter_context(tc.tile_pool(name="ps_g", bufs=6, space="PSUM"))
            ps_s = ctx.enter_context(tc.tile_pool(name="ps_s", bufs=2, space="PSUM"))

            # ------- loads: few fat DMAs, rank-chain data first ------------
            misc_sb = sb.tile([128, 96], mybir.dt.uint32, tag="misc")
            nc.sync.dma_start(
                out=misc_sb[:],
                in_=misc.ap().rearrange("x (p f) -> p x f", p=128),
            )
            cls_sb = misc_sb[:, 0:32].bitcast(I32)
            uscore = misc_sb[:, 32:64].bitcast(F32)
            ucls = misc_sb[:, 64:96].bitcast(I32)
            cmerged = const_pool.tile([128, 384], mybir.dt.uint32, tag="cmerged")
            nc.scalar.dma_start(out=cmerged[:], in_=const_merged.ap())
            tri_sb = cmerged[:, 0:128].bitcast(F32)
            iota_sb = cmerged[:, 128:192].bitcast(BF16)
            sel_sb = cmerged[0:nhalf, 192:320].bitcast(F32)
            sel16_sb = cmerged[0:nhalf, 320:384].bitcast(BF16)
            ghl = sb.tile([nhalf, 2 * GW], BF16, tag="ghl")
            nc.sync.dma_start(
                out=ghl[:], in_=gt_hl.ap().flatten().rearrange("(q x) -> q x", q=nhalf)
            )
            grow = sb.tile([nhalf, GW], F32, tag="grow")
            nc.scalar.dma_start(
                out=grow[:],
                in_=gt_boxes.ap().flatten().rearrange("(q x) -> q x", q=nhalf),
            )
            pbhl = sb.tile([128, 256], BF16, tag="pbhl")
            nc.sync.dma_start(
                out=pbhl[:], in_=pb_hl.ap().flatten().rearrange("(p f) -> p f", p=128)
            )

            # ------- per-block gt areas on GpSimd (only needs grow) ---------
            ag_row = sb.tile([nhalf, M // nhalf], F32, tag="ag_row")
            for blk in range(nblk):
                g0 = 4 * FDB * blk
                a0 = FDB * blk
                wg_r = wrk.tile([nhalf, FDB], F32, tag="wg_r")
                nc.gpsimd.tensor_sub(
                    wg_r[:], grow[:, g0 + 2 : g0 + 4 * FDB : 4],
                    grow[:, g0 + 0 : g0 + 4 * FDB : 4],
                )
                hg_r = wrk.tile([nhalf, FDB], F32, tag="hg_r")
                nc.gpsimd.tensor_sub(
                    hg_r[:], grow[:, g0 + 3 : g0 + 4 * FDB : 4],
                    grow[:, g0 + 1 : g0 + 4 * FDB : 4],
                )
                nc.gpsimd.tensor_mul(ag_row[:, a0 : a0 + FDB], wg_r[:], hg_r[:])

            # ---------------- person mask + ranks ----------------
            m = small.tile([128, 32], F32, tag="m")
            nc.vector.tensor_scalar(m[:], cls_sb[:], 0, None, Alu.is_equal)
            s = small.tile([128, 32], F32, tag="s")
            nc.vector.tensor_tensor_scan(s[:], m[:], m[:], 0.0, Alu.add, Alu.max)
            pref_ps = ps_s.tile([128, 4], F32, tag="pss")
            nc.tensor.matmul(
                pref_ps[:, 0:1], tri_sb, s[:, 31:32], start=True, stop=True
            )
            q = small.tile([128, 32], F32, tag="q")
            nc.vector.scalar_tensor_tensor(
                q[:], s[:], pref_ps[:, 0:1], m[:], Alu.add, Alu.mult
            )

            # ---------------- compaction: one-hot + matmul gather -----------
            pc_ps = ps_s.tile([128, 4], F32, tag="pss")
            for f in range(NCH):
                oh = ohp.tile([128, 128], BF16, tag="oh")
                nc.vector.tensor_scalar(
                    oh[:], iota_sb, q[:, f : f + 1], None, Alu.is_equal
                )
                nc.tensor.matmul(
                    pc_ps[:], oh[:], pbhl[:, 8 * f : 8 * f + 4],
                    start=(f == 0), stop=False,
                )
                nc.tensor.matmul(
                    pc_ps[:], oh[:], pbhl[:, 8 * f + 4 : 8 * f + 8],
                    start=False, stop=(f == NCH - 1),
                )
            pc = small.tile([128, 4], F32, tag="pcs")
            nc.vector.tensor_copy(pc[:], pc_ps[:])
            px1, py1, px2, py2 = (pc[:, i : i + 1] for i in range(4))
            wp = small.tile([128, 1], F32, tag="wp")
            nc.vector.tensor_sub(wp[:], px2, px1)
            hp = small.tile([128, 1], F32, tag="hp")
            nc.vector.tensor_sub(hp[:], py2, py1)
            ap_eps = small.tile([128, 1], F32, tag="ap_eps")
            nc.vector.scalar_tensor_tensor(
                ap_eps[:], wp[:], EPS, hp[:], Alu.bypass, Alu.mult
            )
            nc.vector.tensor_scalar(ap_eps[:], ap_eps[:], EPS, None, Alu.add)

            # ------- gt coord broadcasts (PE, bf16 hi+lo accumulate) --------
            def bcast_coords(blk):
                g0 = blk * 8 * FDB
                tiles = []
                for c in (0, 2, 1, 3):  # x1, x2, y1, y2
                    gt_ps = ps_g.tile([128, FDB], F32, tag="g")
                    nc.tensor.matmul(
                        gt_ps[:], sel16_sb, ghl[:, g0 + c : g0 + 8 * FDB : 8],
                        start=True, stop=False,
                    )
                    nc.tensor.matmul(
                        gt_ps[:], sel16_sb, ghl[:, g0 + c + 4 : g0 + 8 * FDB : 8],
                        start=False, stop=True,
                    )
                    tiles.append(gt_ps)
                return tiles  # [x1, x2, y1, y2]

            def bcast_area(blk):
                ag_ps = ps_g.tile([128, FDB], F32, tag="g")
                nc.tensor.matmul(
                    ag_ps[:], sel_sb, ag_row[:, blk * FDB : (blk + 1) * FDB],
                    start=True, stop=True,
                )
                return ag_ps

            blk_tiles = {0: (bcast_coords(0), bcast_area(0))}

            # ---------------- union max ----------------
            mu = small.tile([128, U // 128], I32, tag="mu")
            nc.vector.tensor_scalar(mu[:], ucls[:], 0, None, Alu.is_equal)
            um = small.tile([128, U // 128], F32, tag="um")
            nc.vector.memset(um[:], -BIG)
            nc.vector.copy_predicated(um[:], mu[:], uscore[:])
            # ---------------- per gt-block pairwise ----------------
            r_all = sb.tile([128, nblk * FDB], F32, tag="r_all")
            for blk in range(nblk):
                (gx1_ps, gx2_ps, gy1_ps, gy2_ps), ag_ps = blk_tiles[blk]
                # prefetch next block's broadcasts onto the PE queue now
                if blk + 1 < nblk:
                    blk_tiles[blk + 1] = (bcast_coords(blk + 1), bcast_area(blk + 1))

                m2x = wrk.tile([128, FDB], F32, tag="m2x")
                nc.vector.tensor_scalar(m2x[:], gx1_ps[:], px1, None, Alu.max)
                zx = wrk.tile([128, FDB], F32, tag="zx")
                nc.vector.scalar_tensor_tensor(
                    zx[:], gx2_ps[:], px2, m2x[:], Alu.min, Alu.subtract
                )
                m2y = wrk.tile([128, FDB], F32, tag="m2y")
                nc.vector.tensor_scalar(m2y[:], gy1_ps[:], py1, None, Alu.max)
                zy = wrk.tile([128, FDB], F32, tag="zy")
                nc.vector.scalar_tensor_tensor(
                    zy[:], gy2_ps[:], py2, m2y[:], Alu.min, Alu.subtract
                )
                ihr = wrk.tile([128, FDB], F32, tag="ihr")
                nc.scalar.activation(ihr[:], zy[:], Act.Relu)
                S_sb = wrk.tile([128, FDB], F32, tag="S")
                nc.scalar.activation(
                    S_sb[:], ag_ps[:], Act.Identity, bias=ap_eps[:], scale=1.0
                )
                srecip = wrk.tile([128, FDB], F32, tag="srecip")
                nc.vector.reciprocal_approx_fast(srecip[:], S_sb[:])
                inter = wrk.tile([128, FDB], F32, tag="inter")
                nc.vector.scalar_tensor_tensor(
                    inter[:], zx[:], 0.0, ihr[:], Alu.max, Alu.mult
                )
                nc.vector.tensor_mul(
                    r_all[:, blk * FDB : (blk + 1) * FDB], inter[:], srecip[:]
                )

            fin = small.tile([128, 2], F32, tag="fin")
            nc.vector.tensor_reduce(fin[:, 0:1], um[:], mybir.AxisListType.X, Alu.max)
            rmax_prev = small.tile([128, 1], F32, tag="rmaxall")
            nc.vector.tensor_reduce(
                rmax_prev[:], r_all[:], mybir.AxisListType.X, Alu.max
            )

            # ---------------- final: iou = r/(1-r) per partition, one
            # fused cross-partition max over [umax | iou] ----------------
            one_m = small.tile([128, 1], F32, tag="one_m")
            nc.vector.tensor_scalar(
                one_m[:], rmax_prev[:], 1.0, -1.0, Alu.subtract, Alu.mult
            )
            rec = small.tile([128, 1], F32, tag="rec")
            nc.vector.reciprocal(rec[:], one_m[:])
            nc.vector.tensor_mul(fin[:, 1:2], rmax_prev[:], rec[:])
            fin_g = small.tile([128, 2], F32, tag="fin_g")
            nc.gpsimd.partition_all_reduce(
                fin_g[:], fin[:], 128, bass_isa.ReduceOp.max
            )
            nc.sync.dma_start(out=out.ap(), in_=fin_g[0:1, :])

    nc.compile()
    return nc


_KERNEL_CACHE = {}

# test/dev hooks
TRACE = False
LAST_RESULTS = None


def _get_kernel(K: int):
    if K not in _KERNEL_CACHE:
        _KERNEL_CACHE[K] = build_kernel(K)
    return _KERNEL_CACHE[K]


def make_in_maps(pred_boxes, pred_classes, gt_boxes, union_scores, union_classes):
    misc_shared = np.stack(
        [
            np.zeros(U, np.uint32),  # per-image, filled below
            union_scores.view(np.uint32),
            union_classes.view(np.uint32),
        ]
    )
    in_maps = []
    for b in range(B):
        ghi, glo = split_hi_lo(gt_boxes[b])
        phi, plo = split_hi_lo(pred_boxes[b])
        misc = misc_shared.copy()
        misc[0] = pred_classes[b].view(np.uint32)
        in_maps.append(
            {
                "misc": misc,
                "pb_hl": np.concatenate([phi, plo], axis=1),
                "gt_boxes": gt_boxes[b],
                "gt_hl": np.concatenate([ghi, glo], axis=1),
            }
        )
    return in_maps


def kernel(pred_boxes, pred_scores, pred_classes, gt_boxes, union_scores, union_classes):
    from concourse.bass_utils import run_bass_kernel_spmd

    pred_boxes = np.ascontiguousarray(np.asarray(pred_boxes, dtype=np.float32))
    pred_classes = np.ascontiguousarray(np.asarray(pred_classes, dtype=np.int32))
    gt_boxes = np.ascontiguousarray(np.asarray(gt_boxes, dtype=np.float32))
    union_scores = np.ascontiguousarray(np.asarray(union_scores, dtype=np.float32))
    union_classes = np.ascontiguousarray(np.asarray(union_classes, dtype=np.int32))

    max_persons = int((pred_classes == 0).sum(axis=1).max())
    K = 64 if max_persons <= 64 else 128
    nc = _get_kernel(K)

    in_maps = make_in_maps(pred_boxes, pred_classes, gt_boxes, union_scores, union_classes)
    res = run_bass_kernel_spmd(nc, in_maps, list(range(B)), trace=TRACE)
    global LAST_RESULTS
    LAST_RESULTS = res
    outs = np.stack([res.results[b]["out"] for b in range(B)])  # [B, 2]
    max_prob = outs[0, 0]
    max_iou = outs[:, 1].mean(dtype=np.float32)
    return np.array([max_prob, max_iou], dtype=np.float32)



# revision 17
# speedup vs baseline: 1.2997x; 1.2997x over previous
"""Trainium2 Bass kernel for nn_MaxExtractor (masked pairwise-IoU max + union max).

Contract: kernel(**inputs) takes FULL unsharded inputs, returns the FULL [2]
output. Internally shards the batch dim (8 images) across 8 NeuronCores, one
image per core; each core computes [max_prob_t, r_max] where r = inter/S is a
monotone transform of IoU (iou = r/(1-r)); the host applies the transform,
guards empty-person images, and averages.

Per-core design (K person slots x nhalf gt-halves = 128 partitions):
  Host compacts person preds (class==0) and valid gt rows, then ships PE
  matmul operands: data rows = gt coords as bf16 hi+lo pairs plus ones-rows,
  weights = half-selectors plus per-person offsets (-px1 / -py1 / +area_p) as
  bf16 hi+lo rows.  One 1-cycle/row bf16 matmul per tile then reconstructs
  exact-f32 broadcast tiles directly shifted per person:
     ux = gx2 - px1, vx = gx1 - px1, uy = gy2 - py1, vy = gy1 - py1,
     S = area_g + area_p
  using iw = min(gx2,px2) - max(gx1,px1) = min(ux, wp) - relu(vx).
  The unary relu runs on the Activation engine (the only engine shape it can
  absorb), min/clamp on Pool, and the fp16 subtract / inter / fused
  divide+max-reduce (tensor_tensor_reduce, running accumulator across blocks)
  on DVE.  PE is warmed up with dummy matmuls during the DMA wait so the
  broadcast matmuls run at full clock (p-state ramp).
"""

import sys

sys.path.insert(0, "/opt/trn_rl_repo")

import contextlib

import numpy as np

import concourse.bacc as bacc
import concourse.mybir as mybir
from concourse import bass_isa
from concourse.tile import TileContext

F32 = mybir.dt.float32
BF16 = mybir.dt.bfloat16
FP16 = mybir.dt.float16
I32 = mybir.dt.int32
Alu = mybir.AluOpType
Act = mybir.ActivationFunctionType

N = 4096  # preds per image
M = 2048  # gts per image
B = 8  # images == cores
U = 4096  # union entries
FDB = 512  # max gt-block free size (1 PSUM bank)
USE_DIV = True  # fused divide in tensor_tensor_reduce (fallback: reciprocal)
NWARM = 5  # PE warmup matmuls


def _cfg(K):
    """Layout constants for a K-person-slot build."""
    nhalf = 128 // K  # gt halves packed along partitions
    mpad = 1920 if nhalf == 2 else 1920  # compacted+padded gt count
    mh = mpad // nhalf  # gts per half
    widths = []
    c = mh
    while c > 0:
        widths.append(min(FDB, c))
        c -= FDB
    nrow = 2 * nhalf + 2  # matmul contract rows: gt hi/lo per half + 2 ones
    return nhalf, mpad, mh, widths, nrow


def split_hi_lo(x):
    bf16 = mybir.dt.np(BF16)
    hi = x.astype(bf16)
    lo = (x.astype(np.float32) - hi.astype(np.float32)).astype(bf16)
    return hi, lo


def build_kernel(K: int):
    assert K in (64, 128)
    nhalf, mpad, mh, widths, nrow = _cfg(K)
    nblk = len(widths)
    # ped column layout: nblk*5 data slices then 3 weight blocks
    doffs = []
    c = 0
    for w in widths:
        doffs.append(c)
        c += 5 * w
    woff = c
    ped_cols = woff + 5 * 128

    nc = bacc.Bacc("TRN2", target_bir_lowering=False, debug=False)

    ped = nc.dram_tensor("ped", [nrow, ped_cols], BF16, kind="ExternalInput")
    scal = nc.dram_tensor("scal", [128, 68], mybir.dt.uint32, kind="ExternalInput")
    out = nc.dram_tensor("out", [2], F32, kind="ExternalOutput")

    with TileContext(nc) as tc:
        ctx = contextlib.ExitStack()
        with ctx:
            sb = ctx.enter_context(tc.tile_pool(name="sbuf", bufs=1))
            wrk = ctx.enter_context(tc.tile_pool(name="wrk", bufs=6))
            rjp = ctx.enter_context(tc.tile_pool(name="rjp", bufs=2))
            small = ctx.enter_context(tc.tile_pool(name="small", bufs=1))
            ps = ctx.enter_context(tc.tile_pool(name="ps", bufs=7, space="PSUM"))
            psw = ctx.enter_context(tc.tile_pool(name="psw", bufs=1, space="PSUM"))

            # warmup operands (constant, no DMA dependency)
            wdat = small.tile([1, FDB], BF16, tag="wdat")
            nc.gpsimd.memset(wdat[:], 1.0)
            wwt = small.tile([1, 128], BF16, tag="wwt")
            nc.gpsimd.memset(wwt[:], 1.0)

            # inputs: ped first (PE chain is longest), scal second
            ped_sb = sb.tile([nrow, ped_cols], BF16, tag="ped")
            nc.sync.dma_start(out=ped_sb[:], in_=ped.ap())
            scal_sb = sb.tile([128, 68], mybir.dt.uint32, tag="scal")
            nc.sync.dma_start(out=scal_sb[:], in_=scal.ap())
            wp = scal_sb[:, 0:1].bitcast(F32)
            hp = scal_sb[:, 1:2].bitcast(F32)
            uscore = scal_sb[:, 4:36].bitcast(F32)
            ucls = scal_sb[:, 36:68].bitcast(I32)
            wts = [ped_sb[:, woff + i * 128 : woff + (i + 1) * 128] for i in range(5)]

            # PE warmup: p-state ramp while DMAs are in flight
            warm_ps = psw.tile([128, FDB], F32, tag="warm")
            for _ in range(NWARM):
                nc.tensor.matmul(warm_ps[:], wwt[:], wdat[:], start=True, stop=True)

            # broadcast matmuls per block: e_x, f_x, e_y, f_y, S
            blk_ps = []
            for b, w in enumerate(widths):
                o = doffs[b]
                tiles = []
                for i in range(5):
                    t = ps.tile([128, FDB], F32, tag="g", name=f"g{b}_{i}")
                    nc.tensor.matmul(
                        t[:, :w], wts[i], ped_sb[:, o + i * w : o + (i + 1) * w],
                        start=True, stop=True,
                    )
                    tiles.append(t)
                blk_ps.append(tiles)

            # pairwise.  e/f form (PE emits e_x = px2-gx2, f_x = gx1-px1, ...):
            #   iw = wp - relu(e_x) - relu(f_x)
            #   niwc = min((relu(e_x)+relu(f_x)) - wp, 0) = -relu(iw)
            #   njh  = (relu(e_y)+relu(f_y)) - hp = -ih  (unclamped)
            #   inter = niwc * njh = relu(iw) * ih   (negatives lose the max)
            #   r = inter * (1/S), fused running max via tensor_tensor_reduce
            fin = small.tile([128, 2], F32, tag="fin")
            rel = []
            for b, w in enumerate(widths):
                ex, fx, ey, fy, S = blk_ps[b]
                rex = wrk.tile([128, FDB], FP16, tag="rex", name=f"rex{b}")
                nc.scalar.activation(rex[:, :w], ex[:, :w], Act.Relu)
                rfx = wrk.tile([128, FDB], FP16, tag="rfx", name=f"rfx{b}")
                nc.scalar.activation(rfx[:, :w], fx[:, :w], Act.Relu)
                rey = wrk.tile([128, FDB], FP16, tag="rey", name=f"rey{b}")
                nc.scalar.activation(rey[:, :w], ey[:, :w], Act.Relu)
                rfy = wrk.tile([128, FDB], FP16, tag="rfy", name=f"rfy{b}")
                nc.vector.tensor_scalar(rfy[:, :w], fy[:, :w], 0.0, None, Alu.max)
                sr = wrk.tile([128, FDB], F32, tag="sr", name=f"sr{b}")
                nc.vector.reciprocal_approx_fast(sr[:, :w], S[:, :w])
                rel.append((rex, rfx, rey, rfy, sr))
            mids = []
            for b, w in enumerate(widths):
                rex, rfx, rey, rfy, sr = rel[b]
                sx = wrk.tile([128, FDB], FP16, tag="sx", name=f"sx{b}")
                nc.gpsimd.tensor_tensor(sx[:, :w], rex[:, :w], rfx[:, :w], Alu.add)
                sy = wrk.tile([128, FDB], FP16, tag="sy", name=f"sy{b}")
                nc.gpsimd.tensor_tensor(sy[:, :w], rey[:, :w], rfy[:, :w], Alu.add)
                mids.append((sx, sy))
            rbt = small.tile([128, nblk], F32, tag="rbt")
            for b, w in enumerate(widths):
                sx, sy = mids[b]
                sr = rel[b][4]
                niwc = wrk.tile([128, FDB], FP16, tag="niwc", name=f"niwc{b}")
                nc.vector.tensor_scalar(
                    niwc[:, :w], sx[:, :w], wp, 0.0, Alu.subtract, Alu.min
                )
                njh = wrk.tile([128, FDB], FP16, tag="njh", name=f"njh{b}")
                nc.vector.tensor_scalar(njh[:, :w], sy[:, :w], hp, None, Alu.subtract)
                inter = wrk.tile([128, FDB], FP16, tag="inter", name=f"inter{b}")
                nc.gpsimd.tensor_tensor(
                    inter[:, :w], niwc[:, :w], njh[:, :w], Alu.mult
                )
                rv = rjp.tile([128, FDB], FP16, tag="rv", name=f"rv{b}")
                nc.vector.tensor_tensor(rv[:, :w], inter[:, :w], sr[:, :w], Alu.mult)
                nc.vector.tensor_reduce(
                    rbt[:, b : b + 1], rv[:, :w], mybir.AxisListType.X, Alu.max
                )
            nc.vector.tensor_reduce(
                fin[:, 1:2], rbt[:], mybir.AxisListType.X, Alu.max
            )

            # union max (scores >= 0 and at least one class-0 entry exists,
            # so masked-multiply + max is exact)
            mu = small.tile([128, 32], F32, tag="mu")
            nc.vector.tensor_scalar(mu[:], ucls[:], 0, None, Alu.is_equal)
            um = small.tile([128, 32], F32, tag="um")
            nc.gpsimd.tensor_tensor(um[:], mu[:], uscore[:], Alu.mult)
            nc.vector.tensor_reduce(fin[:, 0:1], um[:], mybir.AxisListType.X, Alu.max)

            fing = small.tile([128, 2], F32, tag="fing")
            nc.gpsimd.partition_all_reduce(
                fing[:], fin[:], 128, bass_isa.ReduceOp.max
            )
            nc.sync.dma_start(out=out.ap(), in_=fing[0:1, :])

    nc.compile()
    return nc


_KERNEL_CACHE = {}

# test/dev hooks
TRACE = False
LAST_RESULTS = None


def _get_kernel(K: int):
    if K not in _KERNEL_CACHE:
        _KERNEL_CACHE[K] = build_kernel(K)
    return _KERNEL_CACHE[K]


def make_in_maps(pred_boxes, pred_classes, gt_boxes, union_scores, union_classes, K):
    nhalf, mpad, mh, widths, nrow = _cfg(K)
    nblk = len(widths)
    doffs = []
    c = 0
    for w in widths:
        doffs.append(c)
        c += 5 * w
    woff = c
    ped_cols = woff + 5 * 128
    bf16 = mybir.dt.np(BF16)

    # shared union payload
    scal_u = np.zeros((128, 68), np.uint32)
    scal_u[:, 4:36] = union_scores.astype(np.float32).reshape(128, 32).view(np.uint32)
    scal_u[:, 36:68] = union_classes.astype(np.int32).reshape(128, 32).view(np.uint32)

    in_maps = []
    has_person = []
    for b in range(B):
        idx = np.flatnonzero(pred_classes[b] == 0)
        has_person.append(len(idx) > 0)
        p = np.zeros((K, 4), np.float32)
        p[: len(idx)] = pred_boxes[b][idx]
        p = np.tile(p, (nhalf, 1))  # [128, 4]
        px1, py1, px2, py2 = p[:, 0], p[:, 1], p[:, 2], p[:, 3]
        wp = px2 - px1
        hp = py2 - py1
        ap = wp * hp
        # pad persons: ap=1 keeps S >= 1 (their inter is <= 0 so r <= 0)
        padmask = np.tile(np.arange(K) >= len(idx), nhalf)
        ap = np.where(padmask, 1.0, ap).astype(np.float32)

        gv = gt_boxes[b][gt_boxes[b].sum(axis=-1) != 0]
        assert len(gv) <= mpad, f"too many valid gts: {len(gv)}"
        g = np.zeros((mpad, 4), np.float32)
        g[: len(gv)] = gv
        gx1, gy1, gx2, gy2 = g[:, 0], g[:, 1], g[:, 2], g[:, 3]
        ag = ((gx2 - gx1) * (gy2 - gy1)).astype(np.float32)

        ped = np.zeros((nrow, ped_cols), bf16)
        for blk, w in enumerate(widths):
            o = doffs[blk]
            for i, arr in enumerate((-gx2, gx1, -gy2, gy1, ag)):
                sl = ped[:, o + i * w : o + (i + 1) * w]
                for h in range(nhalf):
                    seg = arr[h * mh + blk * FDB : h * mh + blk * FDB + w]
                    hi, lo = split_hi_lo(seg)
                    sl[2 * h] = hi
                    sl[2 * h + 1] = lo
                sl[2 * nhalf] = 1.0
                sl[2 * nhalf + 1] = 1.0

        def wset(off, vec):
            wt = np.zeros((nrow, 128), np.float32)
            for h in range(nhalf):
                wt[2 * h] = wt[2 * h + 1] = (np.arange(128) // K) == h
            hi, lo = split_hi_lo(vec.astype(np.float32))
            wt[2 * nhalf] = hi.astype(np.float32)
            wt[2 * nhalf + 1] = lo.astype(np.float32)
            ped[:, off : off + 128] = wt.astype(bf16)

        # e_x = px2 - gx2, f_x = gx1 - px1, e_y = py2 - gy2, f_y = gy1 - py1
        wset(woff, px2)
        wset(woff + 128, -px1)
        wset(woff + 256, py2)
        wset(woff + 384, -py1)
        wset(woff + 512, ap)

        scal_b = scal_u.copy()
        scal_b[:, 0] = wp.astype(np.float32).view(np.uint32)
        scal_b[:, 1] = hp.astype(np.float32).view(np.uint32)
        in_maps.append({"ped": ped, "scal": scal_b})
    return in_maps, has_person


def kernel(pred_boxes, pred_scores, pred_classes, gt_boxes, union_scores, union_classes):
    from concourse.bass_utils import run_bass_kernel_spmd

    pred_boxes = np.ascontiguousarray(np.asarray(pred_boxes, dtype=np.float32))
    pred_classes = np.ascontiguousarray(np.asarray(pred_classes, dtype=np.int32))
    gt_boxes = np.ascontiguousarray(np.asarray(gt_boxes, dtype=np.float32))
    union_scores = np.ascontiguousarray(np.asarray(union_scores, dtype=np.float32))
    union_classes = np.ascontiguousarray(np.asarray(union_classes, dtype=np.int32))

    max_persons = int((pred_classes == 0).sum(axis=1).max())
    K = 64 if max_persons <= 64 else 128
    nc = _get_kernel(K)

    in_maps, has_person = make_in_maps(
        pred_boxes, pred_classes, gt_boxes, union_scores, union_classes, K
    )
    res = run_bass_kernel_spmd(nc, in_maps, list(range(B)), trace=TRACE)
    global LAST_RESULTS
    LAST_RESULTS = res
    outs = np.stack([res.results[b]["out"] for b in range(B)])  # [B, 2]
    max_prob = outs[0, 0]
    r = np.maximum(outs[:, 1], 0.0)
    iou = r / np.maximum(1.0 - r, 1e-9)
    iou = np.where(np.array(has_person), iou, 0.0)
    max_iou = iou.mean(dtype=np.float32)
    return np.array([max_prob, max_iou], dtype=np.float32)


# revision 22
# speedup vs baseline: 1.4515x; 1.1168x over previous
"""Trainium2 Bass kernel for nn_MaxExtractor (masked pairwise-IoU max + union max).

Contract: kernel(**inputs) takes FULL unsharded inputs, returns the FULL [2]
output. Internally shards the batch dim (8 images) across 8 NeuronCores, one
image per core; each core computes per-partition maxima of r = inter/S (a
monotone transform of IoU: iou = r/(1-r)) plus the union-score max; the host
finishes the cross-partition max, the transform, and the mean.

Per-core design (K person slots x nhalf gt-halves = 128 partitions):
  Host compacts person preds (class==0) and valid gt rows. PE "super"
  matmuls (bf16 hi/lo pair rows, exact f32) broadcast, per block, a
  [128, 2w] x-tile [e_x | f_x] = [px2-gx2 | gx1-px1] (per-column-region
  ones-rows select +px2 vs -px1 weights), the analogous y-tile, and
  S = area_g + area_p.  Then:
    Act:  relu of each [128, 2w] tile -> fp16      (only engine shape that
                                                    can absorb PSUM reads)
    Pool: sx = relu(e_x) + relu(f_x)  (tensor_tensor add)
    DVE:  sr = 1/S (reciprocal_approx_fast)
          niwc = min(sx - wp, 0) = -relu(iw)   (4x fp16 tensor_scalar)
          njh  = sy - hp = -ih
          inter = niwc * njh = relu(iw) * ih   (negatives lose the max)
          rv = inter * sr;  per-block free-dim max -> rbt column
  PE is warmed up with dummy matmuls during the DMA wait (p-state ramp).
  Output is the [128, 4] rbt tile (umax | r per block); host reduces.
"""

import sys

sys.path.insert(0, "/opt/trn_rl_repo")

import contextlib

import numpy as np

import concourse.bacc as bacc
import concourse.mybir as mybir
from concourse.tile import TileContext

F32 = mybir.dt.float32
BF16 = mybir.dt.bfloat16
FP16 = mybir.dt.float16
I32 = mybir.dt.int32
Alu = mybir.AluOpType
Act = mybir.ActivationFunctionType

N = 4096  # preds per image
M = 2048  # gts per image
B = 8  # images == cores
U = 4096  # union entries
FDB = 512  # max gt-block free size (1 PSUM bank)
NWARM = 5  # PE warmup matmuls


def _cfg(K):
    """Layout constants for a K-person-slot build."""
    nhalf = 128 // K  # gt halves packed along partitions
    mpad = M  # zero gt rows are harmless pads; no compaction needed
    mh = mpad // nhalf  # gts per half
    widths = []
    c = mh
    while c > 0:
        widths.append(min(FDB, c))
        c -= FDB
    nrow = 2 * nhalf + 4  # sel rows + two hi/lo ones-row pairs
    return nhalf, mpad, mh, widths, nrow


def split_hi_lo(x):
    bf16 = mybir.dt.np(BF16)
    hi = x.astype(bf16)
    lo = (x.astype(np.float32) - hi.astype(np.float32)).astype(bf16)
    return hi, lo


def _layout(K):
    nhalf, mpad, mh, widths, nrow = _cfg(K)
    doffs = []
    c = 0
    for w in widths:
        doffs.append(c)
        c += 5 * w  # x-pair (2w) + y-pair (2w) + S (w)
    woff = c
    ped_cols = woff + 3 * 128
    return nhalf, mpad, mh, widths, nrow, doffs, woff, ped_cols


def build_kernel(K: int):
    assert K in (64, 128)
    nhalf, mpad, mh, widths, nrow, doffs, woff, ped_cols = _layout(K)
    nblk = len(widths)

    nc = bacc.Bacc("TRN2", target_bir_lowering=False, debug=False)

    ped = nc.dram_tensor("ped", [nrow, ped_cols], BF16, kind="ExternalInput")
    scal = nc.dram_tensor("scal", [128, 68], mybir.dt.uint32, kind="ExternalInput")
    out = nc.dram_tensor("out", [128, 4], F32, kind="ExternalOutput")

    with TileContext(nc) as tc:
        ctx = contextlib.ExitStack()
        with ctx:
            sb = ctx.enter_context(tc.tile_pool(name="sbuf", bufs=1))
            wrk = ctx.enter_context(tc.tile_pool(name="wrk", bufs=4))
            small = ctx.enter_context(tc.tile_pool(name="small", bufs=1))
            ps2 = ctx.enter_context(tc.tile_pool(name="ps2", bufs=2, space="PSUM"))
            ps1 = ctx.enter_context(tc.tile_pool(name="ps1", bufs=2, space="PSUM"))
            psw = ctx.enter_context(tc.tile_pool(name="psw", bufs=1, space="PSUM"))

            # warmup operands first (DVE memsets; no DMA dependency)
            wdat = small.tile([1, FDB], BF16, tag="wdat")
            nc.vector.memset(wdat[:], 1.0)
            wwt = small.tile([1, 128], BF16, tag="wwt")
            nc.vector.memset(wwt[:], 1.0)

            ped_sb = sb.tile([nrow, ped_cols], BF16, tag="ped")
            nc.sync.dma_start(out=ped_sb[:], in_=ped.ap())
            scal_sb = sb.tile([128, 68], mybir.dt.uint32, tag="scal")
            nc.sync.dma_start(out=scal_sb[:], in_=scal.ap())
            wp = scal_sb[:, 0:1].bitcast(F32)
            hp = scal_sb[:, 1:2].bitcast(F32)
            uscore = scal_sb[:, 4:36].bitcast(F32)
            ucls = scal_sb[:, 36:68].bitcast(I32)
            wx_w = ped_sb[:, woff : woff + 128]
            wy_w = ped_sb[:, woff + 128 : woff + 256]
            ws_w = ped_sb[:, woff + 256 : woff + 384]

            # PE warmup: p-state ramp while DMAs are in flight
            warm_ps = psw.tile([128, FDB], F32, tag="warm")
            for _ in range(NWARM):
                nc.tensor.matmul(warm_ps[:], wwt[:], wdat[:], start=True, stop=True)

            # super-matmuls per block: [e_x | f_x], [e_y | f_y], S
            blk_ps = []
            for b, w in enumerate(widths):
                o = doffs[b]
                xt = ps2.tile([128, 2 * FDB], F32, tag="g2", name=f"xt{b}")
                nc.tensor.matmul(
                    xt[:, :w], wx_w, ped_sb[:, o : o + w], start=True, stop=True
                )
                nc.tensor.matmul(
                    xt[:, FDB : FDB + w], wx_w, ped_sb[:, o + w : o + 2 * w],
                    start=True, stop=True,
                )
                yt = ps2.tile([128, 2 * FDB], F32, tag="g2", name=f"yt{b}")
                nc.tensor.matmul(
                    yt[:, :w], wy_w, ped_sb[:, o + 2 * w : o + 3 * w],
                    start=True, stop=True,
                )
                nc.tensor.matmul(
                    yt[:, FDB : FDB + w], wy_w, ped_sb[:, o + 3 * w : o + 4 * w],
                    start=True, stop=True,
                )
                st = ps1.tile([128, FDB], F32, tag="g1", name=f"st{b}")
                nc.tensor.matmul(
                    st[:, :w], ws_w, ped_sb[:, o + 4 * w : o + 5 * w],
                    start=True, stop=True,
                )
                blk_ps.append((xt, yt, st))

            # rbt: col0 = umax, col 1+b = per-block r max (per-partition)
            rbt = small.tile([128, 4], F32, tag="rbt")
            nc.vector.memset(rbt[:, 3:4], 0.0)

            # Act: one wide relu per axis per block (PSUM -> fp16 SBUF)
            rel = []
            for b, w in enumerate(widths):
                xt, yt, st = blk_ps[b]
                rx = wrk.tile([128, 2 * FDB], FP16, tag="rx", name=f"rx{b}")
                nc.scalar.activation(rx[:, : 2 * w], xt[:, : 2 * w], Act.Relu)
                ry = wrk.tile([128, 2 * FDB], FP16, tag="ry", name=f"ry{b}")
                nc.scalar.activation(ry[:, : 2 * w], yt[:, : 2 * w], Act.Relu)
                rel.append((rx, ry))

            # Pool: sx/sy adds; union masked-multiply sits in the gaps
            mu = small.tile([128, 32], F32, tag="mu")
            nc.vector.tensor_scalar(mu[:], ucls[:], 0, None, Alu.is_equal)
            um = small.tile([128, 32], F32, tag="um")
            nc.gpsimd.tensor_tensor(um[:], mu[:], uscore[:], Alu.mult)
            mids = []
            for b, w in enumerate(widths):
                rx, ry = rel[b]
                sx = wrk.tile([128, FDB], FP16, tag="sx", name=f"sx{b}")
                nc.gpsimd.tensor_tensor(
                    sx[:, :w], rx[:, :w], rx[:, w : 2 * w], Alu.add
                )
                sy = wrk.tile([128, FDB], FP16, tag="sy", name=f"sy{b}")
                nc.gpsimd.tensor_tensor(
                    sy[:, :w], ry[:, :w], ry[:, w : 2 * w], Alu.add
                )
                mids.append((sx, sy))

            # DVE chain, ordered by expected data readiness
            nc.vector.tensor_reduce(
                rbt[:, 0:1], um[:], mybir.AxisListType.X, Alu.max
            )
            srs = []
            for b, w in enumerate(widths):
                st = blk_ps[b][2]
                sr = wrk.tile([128, FDB], F32, tag="sr", name=f"sr{b}")
                nc.vector.reciprocal_approx_fast(sr[:, :w], st[:, :w])
                srs.append(sr)
            for b, w in enumerate(widths):
                sx, sy = mids[b]
                sr = srs[b]
                niwc = wrk.tile([128, FDB], FP16, tag="niwc", name=f"niwc{b}")
                nc.vector.tensor_scalar(
                    niwc[:, :w], sx[:, :w], wp, 0.0, Alu.subtract, Alu.min
                )
                njh = wrk.tile([128, FDB], FP16, tag="njh", name=f"njh{b}")
                nc.vector.tensor_scalar(njh[:, :w], sy[:, :w], hp, None, Alu.subtract)
                inter = wrk.tile([128, FDB], FP16, tag="inter", name=f"inter{b}")
                nc.vector.tensor_tensor(
                    inter[:, :w], niwc[:, :w], njh[:, :w], Alu.mult
                )
                rv = wrk.tile([128, FDB], FP16, tag="rv", name=f"rv{b}")
                nc.vector.tensor_tensor(rv[:, :w], inter[:, :w], sr[:, :w], Alu.mult)
                nc.vector.tensor_reduce(
                    rbt[:, 1 + b : 2 + b], rv[:, :w], mybir.AxisListType.X, Alu.max
                )

            nc.sync.dma_start(out=out.ap(), in_=rbt[:])

    nc.compile()
    return nc


_KERNEL_CACHE = {}

# test/dev hooks
TRACE = False
LAST_RESULTS = None


def _get_kernel(K: int):
    if K not in _KERNEL_CACHE:
        _KERNEL_CACHE[K] = build_kernel(K)
    return _KERNEL_CACHE[K]


def make_in_maps(pred_boxes, pred_classes, gt_boxes, union_scores, union_classes, K):
    nhalf, mpad, mh, widths, nrow, doffs, woff, ped_cols = _layout(K)
    bf16 = mybir.dt.np(BF16)

    scal_u = np.zeros((128, 68), np.uint32)
    scal_u[:, 4:36] = union_scores.astype(np.float32).reshape(128, 32).view(np.uint32)
    scal_u[:, 36:68] = union_classes.astype(np.int32).reshape(128, 32).view(np.uint32)

    in_maps = []
    has_person = []
    for b in range(B):
        idx = np.flatnonzero(pred_classes[b] == 0)
        has_person.append(len(idx) > 0)
        p = np.zeros((K, 4), np.float32)
        p[: len(idx)] = pred_boxes[b][idx]
        p = np.tile(p, (nhalf, 1))  # [128, 4]
        px1, py1, px2, py2 = p[:, 0], p[:, 1], p[:, 2], p[:, 3]
        wp = px2 - px1
        hp = py2 - py1
        ap = wp * hp
        # pad persons: ap=1 keeps S >= 1 (their inter is <= 0 so r <= 0)
        padmask = np.tile(np.arange(K) >= len(idx), nhalf)
        ap = np.where(padmask, 1.0, ap).astype(np.float32)

        g = gt_boxes[b]  # zero rows act as pads (their inter contribution <= 0)
        gx1, gy1, gx2, gy2 = g[:, 0], g[:, 1], g[:, 2], g[:, 3]
        ag = ((gx2 - gx1) * (gy2 - gy1)).astype(np.float32)

        ped = np.zeros((nrow, ped_cols), bf16)
        for blk, w in enumerate(widths):
            o = doffs[blk]
            # region columns for this block within each half
            def gseg(arr, h):
                return arr[h * mh + blk * FDB : h * mh + blk * FDB + w]

            # x pair: [-gx2 | gx1], ones rows 4,5 for px2 region, 6,7 for -px1
            for j, (arr, onepair) in enumerate(
                ((-gx2, 0), (gx1, 1), (-gy2, 0), (gy1, 1), (ag, 0))
            ):
                off = o + j * w
                sl = ped[:, off : off + w]
                for h in range(nhalf):
                    hi, lo = split_hi_lo(gseg(arr, h))
                    sl[2 * h] = hi
                    sl[2 * h + 1] = lo
                r0 = 2 * nhalf + 2 * onepair
                sl[r0] = 1.0
                sl[r0 + 1] = 1.0

        def wset(off, vec_a, vec_b):
            # rows: sel(2*nhalf) | hi/lo(vec_a) | hi/lo(vec_b)
            wt = np.zeros((nrow, 128), np.float32)
            for h in range(nhalf):
                wt[2 * h] = wt[2 * h + 1] = (np.arange(128) // K) == h
            ha, la = split_hi_lo(vec_a.astype(np.float32))
            wt[2 * nhalf] = ha.astype(np.float32)
            wt[2 * nhalf + 1] = la.astype(np.float32)
            hb, lb = split_hi_lo(vec_b.astype(np.float32))
            wt[2 * nhalf + 2] = hb.astype(np.float32)
            wt[2 * nhalf + 3] = lb.astype(np.float32)
            ped[:, off : off + 128] = wt.astype(bf16)

        wset(woff, px2, -px1)  # x: e region uses +px2, f region uses -px1
        wset(woff + 128, py2, -py1)
        wset(woff + 256, ap, np.zeros(128))  # S: ag + ap

        scal_b = scal_u.copy()
        scal_b[:, 0] = wp.astype(np.float32).view(np.uint32)
        scal_b[:, 1] = hp.astype(np.float32).view(np.uint32)
        in_maps.append({"ped": ped, "scal": scal_b})
    return in_maps, has_person


def kernel(pred_boxes, pred_scores, pred_classes, gt_boxes, union_scores, union_classes):
    from concourse.bass_utils import run_bass_kernel_spmd

    pred_boxes = np.ascontiguousarray(np.asarray(pred_boxes, dtype=np.float32))
    pred_classes = np.ascontiguousarray(np.asarray(pred_classes, dtype=np.int32))
    gt_boxes = np.ascontiguousarray(np.asarray(gt_boxes, dtype=np.float32))
    union_scores = np.ascontiguousarray(np.asarray(union_scores, dtype=np.float32))
    union_classes = np.ascontiguousarray(np.asarray(union_classes, dtype=np.int32))

    max_persons = int((pred_classes == 0).sum(axis=1).max())
    K = 64 if max_persons <= 64 else 128
    nc = _get_kernel(K)

    in_maps, has_person = make_in_maps(
        pred_boxes, pred_classes, gt_boxes, union_scores, union_classes, K
    )
    res = run_bass_kernel_spmd(nc, in_maps, list(range(B)), trace=TRACE)
    global LAST_RESULTS
    LAST_RESULTS = res
    outs = np.stack([res.results[b]["out"] for b in range(B)])  # [B, 128, 4]
    max_prob = outs[0, :, 0].max()
    r = np.maximum(outs[:, :, 1:3].max(axis=(1, 2)), 0.0)
    iou = r / np.maximum(1.0 - r, 1e-9)
    iou = np.where(np.array(has_person), iou, 0.0)
    max_iou = iou.mean(dtype=np.float32)
    return np.array([max_prob, max_iou], dtype=np.float32)
